# revision 30
# baseline (speedup 1.0000x reference)
"""Trainium2 Bass kernel for nn_DecoderLayer (Mamba block + BitNet FFN).

Sharding: channel-parallel mamba (256 ch/core) -> AllReduce (xproj rows,
chunked by L-halves) -> DVE tensor_tensor_scan over (d,n) lanes, split into
two L-halves with f32 state carry; phase B (out_proj + rmsnorm + BitNet FFN,
fp8 DoubleRow) for half 0 is emitted between the half-1 scan groups so it
executes under them -> each core emits its 256-token slice.
"""
import numpy as np
import ml_dtypes

try:
    import jax
    jax.config.update("jax_compilation_cache_dir", "/root/jaxcache")
    jax.config.update("jax_persistent_cache_min_compile_time_secs", 1.0)
except Exception:
    pass

import concourse.bass as bass
import concourse.mybir as mybir
import concourse.tile as tile
from concourse import bacc
from concourse.bass_utils import run_bass_kernel_spmd

BF16 = mybir.dt.bfloat16
F32 = mybir.dt.float32
FP8 = mybir.dt.float8e4
AF = mybir.ActivationFunctionType
OP = mybir.AluOpType
DR = mybir.MatmulPerfMode.DoubleRow

L, DM, DI, DS, DC, DTR, DFF = 2048, 1024, 2048, 16, 4, 64, 4096
EPS = 1e-6
NCORES = 8
DIC = DI // NCORES   # 256 channels per core
NDT = DIC // 128     # 2 d-tiles
LT = L // NCORES     # 256 tokens per core
NTT = LT // 128      # 2 token-tiles (== halves of L)
LH = L // 2          # 1024 tokens per half
LTH = LT // 2        # 128 tokens per core per half
MAGIC = 12582912.0   # 1.5*2^23: x+M-M == rint(x) for |x|<2^22

_NC_CACHE = {}


def _emit(nc, tc, ctx, g1, g2):
    import contextlib
    RG = [list(range(NCORES))]

    xT = nc.dram_tensor("xT", [DM, L], BF16, kind="ExternalInput")
    x_tok = nc.dram_tensor("x_tok", [LT, DM], BF16, kind="ExternalInput")
    winT = nc.dram_tensor("winT", [DM, 2 * 128 * NDT], BF16, kind="ExternalInput")
    convw = nc.dram_tensor("convw", [DIC, DC], F32, kind="ExternalInput")
    convb = nc.dram_tensor("convb", [DIC, 1], F32, kind="ExternalInput")
    wxpT = nc.dram_tensor("wxpT", [DIC, 96], BF16, kind="ExternalInput")
    wdtT = nc.dram_tensor("wdtT", [DTR, DIC], BF16, kind="ExternalInput")
    bdt = nc.dram_tensor("bdt", [DIC, 1], F32, kind="ExternalInput")
    acol = nc.dram_tensor("acol", [DIC, DS], F32, kind="ExternalInput")
    dpv = nc.dram_tensor("dpv", [DIC, 1], F32, kind="ExternalInput")
    woutT = nc.dram_tensor("woutT", [DI, DM], BF16, kind="ExternalInput")
    n1w = nc.dram_tensor("n1w", [1, DM], BF16, kind="ExternalInput")
    n2w = nc.dram_tensor("n2w", [1, DM], BF16, kind="ExternalInput")
    w1qT = nc.dram_tensor("w1qT", [DM, DFF], FP8, kind="ExternalInput")
    w2qT = nc.dram_tensor("w2qT", [DFF, DM], FP8, kind="ExternalInput")
    out_t = nc.dram_tensor("out", [LT, DM], F32, kind="ExternalOutput")

    singles = ctx.enter_context(tc.tile_pool(name="singles", bufs=1))
    dram = ctx.enter_context(tc.tile_pool(name="dram", bufs=1, space="DRAM"))
    psA_stack = contextlib.ExitStack()
    psum_small = psA_stack.enter_context(
        tc.tile_pool(name="psA", bufs=4, space="PSUM"))
    actpool = ctx.enter_context(tc.tile_pool(name="acts", bufs=1))

    # ---- small per-partition constants
    convw_sb, convb_sb, bdt_sb, acol_sb, dp_sb, carry_sb = [], [], [], [], [], []
    for dt in range(NDT):
        sl = slice(dt * 128, (dt + 1) * 128)
        t1 = singles.tile([128, DC], F32, name=f"cw{dt}")
        nc.sync.dma_start(t1[:, :], convw[sl, :])
        convw_sb.append(t1)
        t2 = singles.tile([128, 1], F32, name=f"cb{dt}")
        nc.sync.dma_start(t2[:, :], convb[sl, :])
        convb_sb.append(t2)
        t3 = singles.tile([128, 1], F32, name=f"bd{dt}")
        nc.sync.dma_start(t3[:, :], bdt[sl, :])
        bdt_sb.append(t3)
        t4 = singles.tile([128, DS], F32, name=f"ac{dt}")
        nc.sync.dma_start(t4[:, :], acol[sl, :])
        acol_sb.append(t4)
        t5 = singles.tile([128, 1], F32, name=f"dp{dt}")
        nc.sync.dma_start(t5[:, :], dpv[sl, :])
        dp_sb.append(t5)
        carry_sb.append(singles.tile([128, DS], F32, name=f"carry{dt}"))
    wxpT_sb = singles.tile([128, NDT, 96], BF16)
    nc.sync.dma_start(wxpT_sb[:, :, :],
                      wxpT.rearrange("(k p) m -> p k m", p=128))
    wdtT_sb = singles.tile([DTR, DIC], BF16)
    nc.sync.dma_start(wdtT_sb[:, :], wdtT[:, :])
    ident_bf = singles.tile([128, 128], BF16)
    from concourse.masks import make_identity
    make_identity(nc, ident_bf[:, :])

    # ================= PHASE A: in_proj (channel-parallel) =================
    conv_stack = contextlib.ExitStack()
    convpool = conv_stack.enter_context(tc.tile_pool(name="convp", bufs=1))
    u_act = [actpool.tile([128, L], BF16, name=f"uact{dt}") for dt in range(NDT)]
    zs = [actpool.tile([128, L], BF16, name=f"zs{dt}") for dt in range(NDT)]
    delta = [actpool.tile([128, L], BF16, name=f"delta{dt}") for dt in range(NDT)]
    du_bf = [actpool.tile([128, L], BF16, name=f"dubf{dt}") for dt in range(NDT)]
    du_p = [actpool.tile([128, L], BF16, name=f"dup{dt}") for dt in range(NDT)]

    with tc.tile_pool(name="init", bufs=1) as init_pool:
        winT_sb = init_pool.tile([128, 8, 2 * 128 * NDT], BF16)
        nc.sync.dma_start(winT_sb[:, :, :],
                          winT.rearrange("(k p) m -> p k m", p=128))
        xT_h = []
        for hh in range(2):
            xh = init_pool.tile([128, 8, LH], BF16, name=f"xTh{hh}")
            nc.sync.dma_start(
                xh[:, :, :],
                xT[:, hh * LH:(hh + 1) * LH].rearrange("(k p) l -> p k l", p=128))
            xT_h.append(xh)

        u_pad = []
        for dt in range(NDT):
            up = convpool.tile([128, L + 3], F32, name=f"upad{dt}")
            nc.vector.memset(up[:, 0:3], 0.0)
            u_pad.append(up)

        # m-tiles: 0..NDT-1 are u chunks, NDT..2*NDT-1 are z chunks
        for mt in range(2 * NDT):
            for c in range(L // 512):
                xh = xT_h[c // 2]
                coff = (c % 2) * 512
                ps = psum_small.tile([128, 512], F32, tag="psA")
                for k in range(8):
                    nc.tensor.matmul(
                        ps[:, :],
                        winT_sb[:, k, mt * 128:(mt + 1) * 128],
                        xh[:, k, coff:coff + 512],
                        start=(k == 0), stop=(k == 7))
                if mt < NDT:
                    nc.scalar.copy(
                        u_pad[mt][:, 3 + c * 512: 3 + (c + 1) * 512], ps[:, :])
                else:
                    nc.scalar.activation(
                        zs[mt - NDT][:, c * 512:(c + 1) * 512], ps[:, :], AF.Silu)

    # conv + silu (u_act bf16), emitted per L-half for early xproj start
    for hh in range(2):
        h0 = hh * LH
        for dt in range(NDT):
            ca = convpool.tile([128, LH], F32, name=f"cva{hh}{dt}", tag="cva")
            cb = convpool.tile([128, LH], F32, name=f"cvb{hh}{dt}", tag="cvb")
            nc.vector.tensor_scalar_mul(ca[:, :], u_pad[dt][:, h0:h0 + LH],
                                        convw_sb[dt][:, 0:1])
            nc.vector.scalar_tensor_tensor(
                cb[:, :], u_pad[dt][:, h0 + 1:h0 + LH + 1],
                convw_sb[dt][:, 1:2], ca[:, :], op0=OP.mult, op1=OP.add)
            nc.vector.scalar_tensor_tensor(
                ca[:, :], u_pad[dt][:, h0 + 2:h0 + LH + 2],
                convw_sb[dt][:, 2:3], cb[:, :], op0=OP.mult, op1=OP.add)
            nc.vector.scalar_tensor_tensor(
                cb[:, :], u_pad[dt][:, h0 + 3:h0 + LH + 3],
                convw_sb[dt][:, 3:4], ca[:, :], op0=OP.mult, op1=OP.add)
            nc.scalar.activation(u_act[dt][:, h0:h0 + LH], cb[:, :], AF.Silu,
                                 bias=convb_sb[dt][:, 0:1])
    conv_stack.close()

    # ===== xproj + AllReduce + delta + du, chunked by L-halves =====
    xp_stack = contextlib.ExitStack()
    xppool = xp_stack.enter_context(tc.tile_pool(name="xpp", bufs=1))
    bcb = dram.tile([32, L], BF16)
    for hh in range(2):
        h0 = hh * LH
        dbl_sb = xppool.tile([96, LH], F32, name=f"dsb{hh}", tag="dsb")
        for c in range(LH // 512):
            ps = psum_small.tile([96, 512], F32, tag="psA")
            for kt in range(NDT):
                nc.tensor.matmul(
                    ps[:, :],
                    wxpT_sb[:, kt, :],
                    u_act[kt][:, h0 + c * 512:h0 + (c + 1) * 512],
                    start=(kt == 0), stop=(kt == NDT - 1))
            stg = xppool.tile([96, 512], F32, name=f"stg{hh}{c}", tag="stg")
            nc.scalar.copy(stg[:, :], ps[:, :])
            ar_i = dram.tile([96, 512], F32, name=f"ari{hh}{c}")
            ar_o = dram.tile([96, 512], F32, name=f"aro{hh}{c}",
                             addr_space="Shared")
            nc.sync.dma_start(ar_i[:, :], stg[:, :])
            nc.gpsimd.collective_compute("AllReduce", OP.add, replica_groups=RG,
                                         ins=[ar_i.opt()], outs=[ar_o.opt()])
            nc.sync.dma_start(dbl_sb[:, c * 512:(c + 1) * 512], ar_o[:, :])
        # B/C rows -> bf16 bounce in DRAM for partition-replication
        bc_bf = xppool.tile([32, LH], BF16, name=f"bc{hh}", tag="bc")
        nc.vector.tensor_copy(bc_bf[:, :], dbl_sb[64:96, :])
        nc.sync.dma_start(bcb[:, h0:h0 + LH], bc_bf[:, :])
        # delta = softplus(wdt @ dt + bdt): exp(x+b) then ln(1+e)
        dt_r = xppool.tile([DTR, LH], BF16, name=f"dtr{hh}", tag="dtr")
        nc.vector.tensor_copy(dt_r[:, :], dbl_sb[0:DTR, :])
        for dt in range(NDT):
            for c in range(LH // 512):
                ps = psum_small.tile([128, 512], F32, tag="psA")
                nc.tensor.matmul(
                    ps[:, :],
                    wdtT_sb[:, dt * 128:(dt + 1) * 128],
                    dt_r[:, c * 512:(c + 1) * 512],
                    start=True, stop=True)
                nc.scalar.activation(delta[dt][:, h0 + c * 512:h0 + (c + 1) * 512],
                                     ps[:, :], AF.Exp, bias=bdt_sb[dt][:, 0:1])
        for dt in range(NDT):
            nc.scalar.activation(delta[dt][:, h0:h0 + LH],
                                 delta[dt][:, h0:h0 + LH], AF.Ln, bias=1.0)
            nc.vector.tensor_tensor(du_bf[dt][:, h0:h0 + LH],
                                    delta[dt][:, h0:h0 + LH],
                                    u_act[dt][:, h0:h0 + LH], op=OP.mult)
            nc.gpsimd.tensor_tensor(du_p[dt][:, h0:h0 + LH],
                                    delta[dt][:, h0:h0 + LH],
                                    u_act[dt][:, h0:h0 + LH], op=OP.mult)
    xp_stack.close()
    psA_stack.close()

    # ================= scan machinery =================
    y_ps_pool = ctx.enter_context(tc.tile_pool(name="yps", bufs=2, space="PSUM"))
    scanp = ctx.enter_context(tc.tile_pool(name="scanp", bufs=2))
    repp = ctx.enter_context(tc.tile_pool(name="repp", bufs=2))
    a2a_i = [[dram.tile([NCORES * 128, LTH], BF16, name=f"a2ai{hh}_{dt}")
              for dt in range(NDT)] for hh in range(2)]
    a2a_o = [[dram.tile([NCORES * 128, LTH], BF16, name=f"a2ao{hh}_{dt}")
              for dt in range(NDT)] for hh in range(2)]
    y_ps_t = {}

    def scan_group(hh, n_lo, n_hi, dts=(0, 1)):
        h0 = hh * LH
        for n in range(n_lo, n_hi):
            brep = repp.tile([128, LH], BF16, name=f"br{hh}_{n}", tag="brep")
            b_src = bcb[n:n + 1, h0:h0 + LH]
            nc.sync.dma_start(brep[:, :], bass.AP(
                tensor=b_src.tensor, offset=b_src.offset,
                ap=[[0, 128]] + [list(p) for p in b_src.ap[1:]]))
            crep = repp.tile([128, LH], BF16, name=f"cr{hh}_{n}", tag="crep")
            c_src = bcb[16 + n:17 + n, h0:h0 + LH]
            nc.sync.dma_start(crep[:, :], bass.AP(
                tensor=c_src.tensor, offset=c_src.offset,
                ap=[[0, 128]] + [list(p) for p in c_src.ap[1:]]))
            for dt in dts:
                if (hh, dt) not in y_ps_t:
                    y_ps_t[(hh, dt)] = y_ps_pool.tile(
                        [128, LH], F32, name=f"yps{hh}_{dt}", tag="yps")
                y_ps = y_ps_t[(hh, dt)]
                dA = scanp.tile([128, LH], BF16, name=f"dA{hh}{dt}{n}", tag="dA")
                nc.scalar.activation(dA[:, :], delta[dt][:, h0:h0 + LH], AF.Exp,
                                     scale=acol_sb[dt][:, n:n + 1])
                dBu = scanp.tile([128, LH], BF16, name=f"dB{hh}{dt}{n}", tag="dBu")
                if n % 4 == 3:
                    nc.gpsimd.tensor_tensor(dBu[:, :], du_p[dt][:, h0:h0 + LH],
                                            brep[:, :], op=OP.mult)
                else:
                    nc.vector.tensor_tensor(dBu[:, :], du_bf[dt][:, h0:h0 + LH],
                                            brep[:, :], op=OP.mult)
                h = scanp.tile([128, LH], BF16, name=f"h{hh}{dt}{n}", tag="h")
                init = 0.0 if hh == 0 else carry_sb[dt][:, n:n + 1]
                nc.vector.tensor_tensor_scan(h[:, :], dA[:, :], dBu[:, :], init,
                                             OP.mult, OP.add)
                if hh == 0:
                    nc.scalar.copy(carry_sb[dt][:, n:n + 1], h[:, LH - 1:LH])
                yt = scanp.tile([128, LH], BF16, name=f"yt{hh}{dt}{n}", tag="yt")
                eng = nc.gpsimd if (n % 2 == 1) else nc.vector
                eng.tensor_tensor(yt[:, :], h[:, :], crep[:, :], op=OP.mult)
                for c in range(LH // 512):
                    nc.tensor.matmul(
                        y_ps[:, c * 512:(c + 1) * 512],
                        ident_bf[:, :],
                        yt[:, c * 512:(c + 1) * 512],
                        start=(n == 0), stop=(n == DS - 1),
                        skip_group_check=True)

    def gate_a2a(hh, dts=(0, 1)):
        h0 = hh * LH
        for dt in dts:
            y_ps = y_ps_t.pop((hh, dt))
            t1 = scanp.tile([128, LH], BF16, name=f"yg{hh}{dt}", tag="yg")
            nc.vector.scalar_tensor_tensor(
                t1[:, :], u_act[dt][:, h0:h0 + LH], dp_sb[dt][:, 0:1], y_ps[:, :],
                op0=OP.mult, op1=OP.add)
            yh = scanp.tile([128, LH], BF16, name=f"yh{hh}{dt}", tag="yhat")
            nc.vector.tensor_tensor(yh[:, :], t1[:, :], zs[dt][:, h0:h0 + LH],
                                    op=OP.mult)
            nc.sync.dma_start(
                a2a_i[hh][dt].rearrange("(j c) t -> c j t", c=128)[:, :, :],
                yh.rearrange("c (j t) -> c j t", j=NCORES))
            nc.gpsimd.collective_compute(
                "AllToAll", OP.bypass, replica_groups=RG,
                ins=[a2a_i[hh][dt].opt()], outs=[a2a_o[hh][dt].opt()])

    # ================= PHASE B pieces (per half hh == token tile) ==========
    bpool = ctx.enter_context(tc.tile_pool(name="bpool", bufs=1))
    wpool = ctx.enter_context(tc.tile_pool(name="wstream", bufs=3))
    psB = ctx.enter_context(tc.tile_pool(name="psB", bufs=4, space="PSUM"))
    x_tok_sb = bpool.tile([128, NTT, DM], BF16)
    nc.sync.dma_start(x_tok_sb[:, :, :],
                      x_tok.rearrange("(tt p) m -> p tt m", p=128))
    n1w_rep = bpool.tile([128, DM], BF16)
    s1 = n1w[0:1, :]
    nc.sync.dma_start(n1w_rep[:, :], bass.AP(
        tensor=s1.tensor, offset=s1.offset,
        ap=[[0, 128]] + [list(p) for p in s1.ap[1:]]))
    n2w_rep = bpool.tile([128, DM], BF16)
    s2 = n2w[0:1, :]
    nc.sync.dma_start(n2w_rep[:, :], bass.AP(
        tensor=s2.tensor, offset=s2.offset,
        ap=[[0, 128]] + [list(p) for p in s2.ap[1:]]))
    woutT_sb = bpool.tile([128, DI // 128, DM], BF16)
    nc.sync.dma_start(woutT_sb[:, :, :],
                      woutT.rearrange("(k p) m -> p k m", p=128))
    w1v = w1qT.rearrange("(k p) j -> p k j", p=128)
    x1_l, scl1_l, xqT_l, f_l, scl2_l, fqT_l = {}, {}, {}, {}, {}, {}

    def outproj_quant1(tt):
        yfs = []
        for dt in range(NDT):
            yf = bpool.tile([128, NCORES, LTH], BF16, name=f"yf{tt}{dt}",
                            tag=f"yf{dt}")
            nc.sync.dma_start(
                yf[:, :, :],
                a2a_o[tt][dt].rearrange("(j p) t -> p j t", p=128))
            yfs.append(yf)
        s = bpool.tile([128, DM], F32, name=f"s{tt}", tag="s")
        for c in range(DM // 512):
            hps = psB.tile([128, 512], F32, tag="ps512")
            for k in range(DI // 128):
                dt, j = k // 8, k % 8
                nc.tensor.matmul(
                    hps[:, :],
                    yfs[dt][:, j, :],
                    woutT_sb[:, j * 2 + dt, c * 512:(c + 1) * 512],
                    start=(k == 0), stop=(k == DI // 128 - 1))
            nc.vector.tensor_tensor(s[:, c * 512:(c + 1) * 512],
                                    x_tok_sb[:, tt, c * 512:(c + 1) * 512],
                                    hps[:, :], op=OP.add)
        sq = bpool.tile([128, DM], F32, name=f"sq{tt}", tag="sqs")
        ssum = bpool.tile([128, 1], F32, name=f"ssum{tt}", tag="ssum")
        nc.scalar.activation(sq[:, :], s[:, :], AF.Square, accum_out=ssum[:, 0:1])
        v = bpool.tile([128, 1], F32, name=f"v{tt}", tag="v")
        nc.vector.tensor_scalar(v[:, :], ssum[:, :], 1.0 / DM, EPS,
                                op0=OP.mult, op1=OP.add)
        nc.scalar.activation(v[:, :], v[:, :], AF.Ln)
        nc.scalar.activation(v[:, :], v[:, :], AF.Exp, scale=-0.5)
        x1 = bpool.tile([128, DM], F32, name=f"x1_{tt}", tag=f"x1_{tt}")
        nc.vector.scalar_tensor_tensor(x1[:, :], s[:, :], v[:, 0:1],
                                       n1w_rep[:, :], op0=OP.mult, op1=OP.mult)
        x1_l[tt] = x1
        amax = bpool.tile([128, 1], F32, name=f"am{tt}", tag="am")
        nc.vector.tensor_reduce(amax[:, :], x1[:, :], axis=mybir.AxisListType.X,
                                op=OP.max, apply_absolute_value=True)
        nc.vector.tensor_scalar(amax[:, :], amax[:, :], 1e-5, None, op0=OP.max)
        sc = bpool.tile([128, 1], F32, name=f"sc{tt}", tag="scq")
        nc.vector.reciprocal(sc[:, :], amax[:, :])
        nc.vector.tensor_scalar(sc[:, :], sc[:, :], 127.0, None, op0=OP.mult)
        scl1 = bpool.tile([128, 1], F32, name=f"scl1_{tt}", tag=f"scl1_{tt}")
        nc.vector.tensor_scalar(scl1[:, :], amax[:, :], g1 / 127.0, None,
                                op0=OP.mult)
        scl1_l[tt] = scl1
        xq8 = bpool.tile([128, DM // 128, 128], FP8, name=f"xq8{tt}",
                         tag=f"xq8_{tt}")
        for c in range(DM // 512):
            q = bpool.tile([128, 512], F32, name=f"q{tt}{c}", tag="q")
            nc.vector.tensor_scalar(q[:, :], x1[:, c * 512:(c + 1) * 512],
                                    sc[:, 0:1], MAGIC, op0=OP.mult, op1=OP.add)
            xq = bpool.tile([128, 512], BF16, name=f"xq{tt}{c}", tag="xq")
            nc.vector.tensor_scalar(xq[:, :], q[:, :], MAGIC, None,
                                    op0=OP.subtract)
            xqT = bpool.tile([128, 4, 128], BF16, name=f"xqT{tt}{c}", tag="xqTb")
            nc.sync.dma_start_transpose(xqT[:, :, :], xq[:, :])
            nc.vector.tensor_copy(xq8[:, c * 4:(c + 1) * 4, :], xqT[:, :, :])
        xqT_l[tt] = xq8

    def ffn1_gelu(tt, jc_lo, jc_hi):
        if tt not in f_l:
            f_l[tt] = bpool.tile([128, DFF], BF16, name=f"f{tt}", tag="f")
        f_sb = f_l[tt]
        for jc in range(jc_lo, jc_hi):
            w1c = wpool.tile([128, 8, 512], FP8, name=f"w1c{tt}{jc}", tag="wst")
            nc.sync.dma_start(w1c[:, :, :],
                              w1v[:, :, jc * 512:(jc + 1) * 512])
            fps = psB.tile([128, 512], F32, tag="ps512")
            for kk in range(DM // 256):
                nc.tensor.matmul(
                    fps[:, :], xqT_l[tt][:, 2 * kk:2 * kk + 2, :],
                    w1c[:, 2 * kk:2 * kk + 2, :],
                    start=(kk == 0), stop=(kk == DM // 256 - 1),
                    perf_mode=DR)
            nc.scalar.activation(f_sb[:, jc * 512:(jc + 1) * 512], fps[:, :],
                                 AF.Gelu_apprx_tanh, scale=scl1_l[tt][:, 0:1])

    def quant2(tt):
        f_sb = f_l[tt]
        amax2 = bpool.tile([128, 1], F32, name=f"am2{tt}", tag="am2")
        nc.vector.tensor_reduce(amax2[:, :], f_sb[:, :], axis=mybir.AxisListType.X,
                                op=OP.max, apply_absolute_value=True)
        nc.vector.tensor_scalar(amax2[:, :], amax2[:, :], 1e-5, None, op0=OP.max)
        sc2 = bpool.tile([128, 1], F32, name=f"sc2{tt}", tag="sc2q")
        nc.vector.reciprocal(sc2[:, :], amax2[:, :])
        nc.vector.tensor_scalar(sc2[:, :], sc2[:, :], 127.0, None, op0=OP.mult)
        scl2 = bpool.tile([128, 1], F32, name=f"scl2_{tt}", tag=f"scl2_{tt}")
        nc.vector.tensor_scalar(scl2[:, :], amax2[:, :], g2 / 127.0, None,
                                op0=OP.mult)
        scl2_l[tt] = scl2
        fq8 = bpool.tile([128, DFF // 128, 128], FP8, name=f"fq8{tt}",
                         tag=f"fq8_{tt}")
        for jc in range(DFF // 1024):
            q2 = bpool.tile([128, 1024], F32, name=f"q2{tt}{jc}", tag="q2")
            nc.vector.tensor_scalar(q2[:, :],
                                    f_sb[:, jc * 1024:(jc + 1) * 1024],
                                    sc2[:, 0:1], MAGIC, op0=OP.mult, op1=OP.add)
            fq = bpool.tile([128, 1024], BF16, name=f"fq{tt}{jc}", tag="fq")
            nc.vector.tensor_scalar(fq[:, :], q2[:, :], MAGIC, None,
                                    op0=OP.subtract)
            fqT = bpool.tile([128, 8, 128], BF16, name=f"fqT{tt}{jc}", tag="fqT")
            nc.sync.dma_start_transpose(fqT[:, :, :], fq[:, :])
            nc.vector.tensor_copy(fq8[:, jc * 8:(jc + 1) * 8, :], fqT[:, :, :])
        fqT_l[tt] = fq8

    w2v = w2qT.rearrange("(k p) m -> p k m", p=128)
    ops_l = {}

    def ffn2_mm(tt):
        ops_l[tt] = []
        for mc in range(DM // 512):
            ops_ = psB.tile([128, 512], F32, tag="ps512")
            for kg in range(4):
                w2c = wpool.tile([128, 8, 512], FP8, name=f"w2c{tt}{mc}{kg}",
                                 tag="wst")
                nc.sync.dma_start(
                    w2c[:, :, :],
                    w2v[:, kg * 8:(kg + 1) * 8, mc * 512:(mc + 1) * 512])
                for kk in range(4):
                    nc.tensor.matmul(
                        ops_[:, :],
                        fqT_l[tt][:, kg * 8 + 2 * kk:kg * 8 + 2 * kk + 2, :],
                        w2c[:, 2 * kk:2 * kk + 2, :],
                        start=(kg == 0 and kk == 0),
                        stop=(kg == 3 and kk == 3),
                        perf_mode=DR)
            ops_l[tt].append(ops_)

    def ffn2_post(tt):
        o2 = bpool.tile([128, DM], F32, name=f"o2{tt}", tag="o2")
        for mc in range(DM // 512):
            nc.vector.scalar_tensor_tensor(
                o2[:, mc * 512:(mc + 1) * 512], ops_l[tt][mc][:, :],
                scl2_l[tt][:, 0:1],
                x1_l[tt][:, mc * 512:(mc + 1) * 512], op0=OP.mult, op1=OP.add)
        sq2 = bpool.tile([128, DM], F32, name=f"sq2{tt}", tag="sqs")
        ssum2 = bpool.tile([128, 1], F32, name=f"ssum2{tt}", tag="ssum2")
        nc.scalar.activation(sq2[:, :], o2[:, :], AF.Square,
                             accum_out=ssum2[:, 0:1])
        v2 = bpool.tile([128, 1], F32, name=f"v2{tt}", tag="v2")
        nc.vector.tensor_scalar(v2[:, :], ssum2[:, :], 1.0 / DM, EPS,
                                op0=OP.mult, op1=OP.add)
        nc.scalar.activation(v2[:, :], v2[:, :], AF.Ln)
        nc.scalar.activation(v2[:, :], v2[:, :], AF.Exp, scale=-0.5)
        ot = bpool.tile([128, DM], F32, name=f"ot{tt}", tag="ot")
        nc.vector.scalar_tensor_tensor(ot[:, :], o2[:, :], v2[:, 0:1],
                                       n2w_rep[:, :], op0=OP.mult, op1=OP.mult)
        nc.sync.dma_start(out_t[tt * 128:(tt + 1) * 128, :], ot[:, :])

    # ================= pipelined emission =================
    scan_group(0, 0, 12)
    scan_group(0, 12, DS, dts=(0,))
    gate_a2a(0, dts=(0,))
    scan_group(0, 12, DS, dts=(1,))
    gate_a2a(0, dts=(1,))
    outproj_quant1(0)
    ffn1_gelu(0, 0, 4)
    scan_group(1, 0, 8)
    ffn1_gelu(0, 4, 8)
    quant2(0)
    scan_group(1, 8, 12)
    ffn2_mm(0)
    scan_group(1, 12, DS, dts=(0,))
    gate_a2a(1, dts=(0,))
    scan_group(1, 12, DS, dts=(1,))
    ffn2_post(0)
    gate_a2a(1, dts=(1,))
    outproj_quant1(1)
    ffn1_gelu(1, 0, 8)
    quant2(1)
    ffn2_mm(1)
    ffn2_post(1)


def build_nc(g1, g2):
    from contextlib import ExitStack
    nc = bacc.Bacc("TRN2", target_bir_lowering=False, debug=False,
                   num_devices=NCORES)
    with ExitStack() as ctx:
        tc = ctx.enter_context(tile.TileContext(nc))
        _emit(nc, tc, ctx, g1, g2)
    nc.compile()
    return nc


def host_prep(inputs):
    bf = ml_dtypes.bfloat16
    f8 = ml_dtypes.float8_e4m3
    x = np.asarray(inputs["x"], np.float32)
    x2d = x.reshape(L, DM)
    w_in = np.asarray(inputs["w_in"], np.float32)
    conv_w = np.asarray(inputs["conv_w"], np.float32)
    conv_b = np.asarray(inputs["conv_b"], np.float32)
    w_xproj = np.asarray(inputs["w_xproj"], np.float32)
    w_dt = np.asarray(inputs["w_dt"], np.float32)
    b_dt = np.asarray(inputs["b_dt"], np.float32)
    A_log = np.asarray(inputs["A_log"], np.float32)
    Dp = np.asarray(inputs["Dp"], np.float32)
    w_out = np.asarray(inputs["w_out"], np.float32)
    n1 = np.asarray(inputs["norm1_w"], np.float32)
    n2 = np.asarray(inputs["norm2_w"], np.float32)
    w1 = np.asarray(inputs["ffn_w1"], np.float32)
    w2 = np.asarray(inputs["ffn_w2"], np.float32)
    b1 = np.asarray(inputs["ffn_b1"], np.float32)
    b2 = np.asarray(inputs["ffn_b2"], np.float32)
    assert np.all(b1 == 0.0) and np.all(b2 == 0.0), "nonzero ffn bias unsupported"

    g1 = float(np.maximum(np.mean(np.abs(w1), dtype=np.float32), 1e-5))
    g2 = float(np.maximum(np.mean(np.abs(w2), dtype=np.float32), 1e-5))
    w1q = np.clip(np.rint(w1 / g1), -1.0, 1.0).astype(np.float32)
    w2q = np.clip(np.rint(w2 / g2), -1.0, 1.0).astype(np.float32)

    xT_bf = np.ascontiguousarray(x2d.T).astype(bf)
    woutT_bf = np.ascontiguousarray(w_out.T).astype(bf)
    w1qT_f8 = np.ascontiguousarray(w1q.T).astype(f8)
    w2qT_f8 = np.ascontiguousarray(w2q.T).astype(f8)
    n1r = np.ascontiguousarray(n1.reshape(1, DM)).astype(bf)
    n2r = np.ascontiguousarray(n2.reshape(1, DM)).astype(bf)
    A = -np.exp(A_log)

    in_maps = []
    for c in range(NCORES):
        ch = slice(c * DIC, (c + 1) * DIC)
        w_sel = np.concatenate([w_in[c * DIC:(c + 1) * DIC],
                                w_in[DI + c * DIC:DI + (c + 1) * DIC]], axis=0)
        # core c owns token tiles [c*128, (c+1)*128) of each L-half
        xtk = np.concatenate([x2d[c * 128:(c + 1) * 128],
                              x2d[LH + c * 128:LH + (c + 1) * 128]], axis=0)
        in_maps.append({
            "xT": xT_bf,
            "x_tok": np.ascontiguousarray(xtk).astype(bf),
            "winT": np.ascontiguousarray(w_sel.T).astype(bf),
            "convw": np.ascontiguousarray(conv_w[ch, 0, :]),
            "convb": np.ascontiguousarray(conv_b[ch].reshape(-1, 1)),
            "wxpT": np.ascontiguousarray(w_xproj[:, ch].T).astype(bf),
            "wdtT": np.ascontiguousarray(w_dt[ch, :].T).astype(bf),
            "bdt": np.ascontiguousarray(b_dt[ch].reshape(-1, 1)),
            "acol": np.ascontiguousarray(A[ch, :]),
            "dpv": np.ascontiguousarray(Dp[ch].reshape(-1, 1)),
            "woutT": woutT_bf,
            "n1w": n1r,
            "n2w": n2r,
            "w1qT": w1qT_f8,
            "w2qT": w2qT_f8,
        })
    return in_maps, g1, g2


def kernel(**inputs) -> np.ndarray:
    in_maps, g1, g2 = host_prep(inputs)
    key = (round(g1, 10), round(g2, 10))
    if key not in _NC_CACHE:
        _NC_CACHE[key] = build_nc(g1, g2)
    nc = _NC_CACHE[key]
    res = run_bass_kernel_spmd(nc, in_maps, core_ids=list(range(NCORES)))
    full = np.empty((L, DM), np.float32)
    for c in range(NCORES):
        o = res.results[c]["out"]
        full[c * 128:(c + 1) * 128] = o[0:128]
        full[LH + c * 128:LH + (c + 1) * 128] = o[128:256]
    return np.ascontiguousarray(full.reshape(1, L, DM))
